# revision 67
# baseline (speedup 1.0000x reference)
"""BiDAF attention kernel for Trainium2, data-parallel over batch on 8 NeuronCores.

Reference math (per batch b):
    S = (ctx * w_m) @ query^T + ctx@w_c [:,None] + query@w_q [None,:]   [C, Q]
    a = softmax(S, axis=q);       attended_query    aq = a @ query       [C, H]
    m = max(S, axis=q); bweights = softmax(m, axis=c)
    attended_context ac = bweights @ ctx                                  [H]
    G = concat([ctx, aq, ctx*aq, ctx*ac[None,:]], axis=-1)               [C, 4H]

The kernel is HBM-bound, so the schedule exists to keep the DMA engines
busy with the minimum possible byte count (18.3 MiB/core = 53.2 us at
the 360 B/ns cost-model rate, vs 20.5 MiB for an all-f32 layout):
  - ctx and query are loaded as BF16 via GpSimd casting DMAs (the DMA
    engine converts inline; cost is charged on the *output* bytes), so
    the loads cost half.  All compute (S matmuls, softmax weights,
    aq/ac matmuls) runs on the bf16 copies; S still accumulates in fp32
    PSUM and the softmax stats/evacuations stay fp32, so the end-to-end
    error is ~1e-3 against the 2e-2 gate.
  - G[:, 0:H] = ctx is echoed DRAM->DRAM straight from the input tensor
    in exact fp32, split into 16 per-c-tile chunks (728 ns each): these
    have zero dependencies, so the scheduler packs them (plus the loads)
    into the entire ~18 us compute ramp-up of batch 0, and the store
    stream takes over without a bubble.
  - All loads are emitted first with per-batch buffers so no load can be
    trapped behind a compute-gated store in an engine queue; each batch
    loads query before ctx (the tiny query starts the rhs_ext chain).
  - bf16 makes every PE matmul and transpose 1 cycle/row (fp32 is 4).
  - W loaded natural [12, 128] (12 line-rate descriptors) + PE transpose.
  - S accumulated in PSUM [128c, 65] per c-tile (two half-batch banks):
    4 matmuls with lhsT=ctxT chunk, rhs=[qT*w_m | w_c], plus one K=1
    matmul with lhsT=ones, rhs=[s_q | 0] -> col 64 holds s_c, cols 0:63
    hold s_m + s_q.  exp on ScalarE with bias=s_c and accum_out giving
    softmax denominators for free (|S| <= ~8, exp safe without max-sub).
    ctx^T is built t-major with per-tile PSUM evacuation so the first S
    tile (hence the first store) has the shortest critical path.
  - a renormalized lazily: aq matmul uses unnormalized exp(S); the PSUM
    evacuation multiplies by 1/rowsum.
  - b-path: em_t = exp(max_q S_t + s_c_t) feeds the ac matmul with a
    stride-0 broadcast lhsT so attended_context lands pre-broadcast as
    [128, H]; the partition sum of em runs on GpSimd.
  - Stores per c-tile: G2 [H:2H] on its own DMA (it reaches the queue
    ~600 ns before the G3 multiply finishes), then G3 [2H:3H], both on
    the SP HWDGE ring; G4 [3H:4H] via GpSimd SWDGE once the batch-global
    attended_context is ready (second issue queue).

Cost-model result: 56571 ns = 1966 fixed preamble/first-issue + 53161
gapless DMA + 1444 sem-prop/epilogue; zero idle between the first and
last transfer.
"""

import numpy as np
from contextlib import ExitStack

import concourse.bass as bass
import concourse.bacc as bacc
import concourse.bass_isa as bass_isa
import concourse.tile as tile
from concourse import mybir
from concourse.bass_utils import run_bass_kernel_spmd
from concourse.masks import make_identity

F32 = mybir.dt.float32
BF16 = mybir.dt.bfloat16
AF = mybir.ActivationFunctionType

B, C, Q, H = 32, 512, 64, 512
NCORES = 8
BPC = B // NCORES  # batches per core
CT = C // 128  # c tiles
KT = H // 128  # contraction chunks


def build_nc():
    nc = bacc.Bacc("TRN2", target_bir_lowering=False, debug=False)
    ctx_d = nc.dram_tensor("context", [BPC, C, H], F32, kind="ExternalInput")
    qry_d = nc.dram_tensor("query", [BPC, Q, H], F32, kind="ExternalInput")
    w_d = nc.dram_tensor("W", [3 * H], F32, kind="ExternalInput")
    g_d = nc.dram_tensor("G", [BPC, C, 4 * H], F32, kind="ExternalOutput")

    with tile.TileContext(nc) as tc, ExitStack() as ex:
        consts = ex.enter_context(tc.tile_pool(name="consts", bufs=1))
        ctx_pool = ex.enter_context(tc.tile_pool(name="ctx", bufs=4))
        ctxT_pool = ex.enter_context(tc.tile_pool(name="ctxT", bufs=2))
        q_pool = ex.enter_context(tc.tile_pool(name="q", bufs=4))
        small_pool = ex.enter_context(tc.tile_pool(name="small", bufs=3))
        g_pool = ex.enter_context(tc.tile_pool(name="g", bufs=8))
        ps_ctxT = ex.enter_context(tc.tile_pool(name="ps_ctxT", bufs=2, space="PSUM"))
        ps_S = ex.enter_context(tc.tile_pool(name="ps_S", bufs=2, space="PSUM"))
        ps_aq = ex.enter_context(tc.tile_pool(name="ps_aq", bufs=1, space="PSUM"))
        ps_small = ex.enter_context(tc.tile_pool(name="ps_small", bufs=2, space="PSUM"))
        ps_b = ex.enter_context(tc.tile_pool(name="ps_b", bufs=1, space="PSUM"))

        # --- constants ---
        wsb = consts.tile([128, 12], F32)  # cols 0:4 w_c, 4:8 w_q, 8:12 w_m chunks
        ident = consts.tile([128, 128], BF16, tag="ident")
        make_identity(nc, ident)
        ones_row = consts.tile([1, 128], BF16)
        nc.vector.memset(ones_row, 1.0)

        def stage_load(b):
            st = {}
            ctx_v = ctx_d[b].rearrange("(t p) d -> p t d", p=128)  # [128, CT, H]
            st["g_v"] = g_v = g_d[b].rearrange("(t p) f -> p t f", p=128)

            def g1(t, g_v=g_v, ctx_v=ctx_v):
                # DRAM->DRAM echo of one c-tile (728 ns): zero-dependency
                # stream filler.  SP-queue position + arrival arbitration pace
                # these between the loads (which are SWDGE descriptor-gen
                # rate-limited) and the stores.
                nc.sync.dma_start(out=g_v[:, t, 0:H], in_=ctx_v[:, t, :])
            st["g1"] = g1
            # bf16 casting loads (GpSimd is the only engine that casts),
            # query before ctx: the tiny query starts the rhs_ext chain that
            # gates the batch's first S matmul.
            ctx_sb = ctx_pool.tile([128, CT, H], BF16, tag="ctx_sb", name=f"ctx_sb{b}")
            q_sb = q_pool.tile([Q, H], BF16, tag="q_sb")
            nc.gpsimd.dma_start(out=q_sb, in_=qry_d[b])
            nc.gpsimd.dma_start(out=ctx_sb, in_=ctx_v)
            if b == 0:
                # W loaded natural [12, 128] (12 descriptors at line rate), then
                # transposed on PE to the [128, 12] per-partition layout.
                wnat = small_pool.tile([12, 128], BF16, tag="wnat")
                nc.gpsimd.dma_start(
                    out=wnat, in_=w_d[:].rearrange("(g p) -> g p", p=128)
                )
                w_ps = ps_small.tile([128, 12], BF16, tag="ps_misc")
                nc.tensor.transpose(w_ps, wnat, ident[:12, :12])
                nc.vector.tensor_copy(wsb, w_ps)
            st["ctx_sb"] = ctx_sb
            st["q_sb"] = q_sb
            return st

        def stage_early(b, st):
            ctx_sb, q_sb = st["ctx_sb"], st["q_sb"]

            # --- query transpose + scaled rhs build ---
            qt_ps = ps_small.tile([128, KT * Q], BF16, tag="ps_misc")
            for k in range(KT):
                nc.tensor.transpose(
                    qt_ps[:, k * Q : (k + 1) * Q],
                    q_sb[:, k * 128 : (k + 1) * 128],
                    ident[:Q, :Q],
                )
            qT_sb = small_pool.tile([128, KT * Q], BF16, tag="qT_sb")
            nc.vector.tensor_copy(qT_sb, qt_ps)

            # rhs_ext[:, k, 0:64] = qT_k * w_m_k ; [:, k, 64] = w_c_k
            rhs_ext = small_pool.tile([128, KT, Q + 1], BF16, tag="rhs_ext")
            for k in range(KT):
                nc.vector.tensor_scalar_mul(
                    out=rhs_ext[:, k, 0:Q],
                    in0=qT_sb[:, k * Q : (k + 1) * Q],
                    scalar1=wsb[:, 8 + k : 9 + k],
                )
                nc.vector.tensor_copy(rhs_ext[:, k, Q : Q + 1], wsb[:, k : k + 1])

            # s_q^T = w_q . qT  -> [1, Q]  (lhsT must be bf16 to match rhs)
            wq_b = small_pool.tile([128, KT], BF16, tag="wq_b")
            nc.vector.tensor_copy(wq_b, wsb[:, 4:8])
            sq_ps = ps_small.tile([1, Q], F32, tag="ps_misc")
            for k in range(KT):
                nc.tensor.matmul(
                    sq_ps,
                    lhsT=wq_b[:, k : k + 1],
                    rhs=qT_sb[:, k * Q : (k + 1) * Q],
                    start=(k == 0),
                    stop=(k == KT - 1),
                )
            rhs_sq = small_pool.tile([1, Q + 1], BF16, tag="rhs_sq")
            nc.vector.memset(rhs_sq, 0.0)
            nc.vector.tensor_copy(rhs_sq[:, 0:Q], sq_ps)

            # --- per c-tile: transpose ctx chunks, S matmuls, softmax stats.
            # t-major transposes + per-tile evacuation put the first S tile
            # (and so the first store of the batch) on the shortest possible
            # critical path instead of waiting for the full ctx^T build. ---
            ctxT_sb = ctxT_pool.tile([128, KT, C], BF16, tag="ctxT_sb")
            s_lo = ps_S.tile([128, 2, Q + 1], F32, tag="ps_S")
            s_hi = ps_S.tile([128, 2, Q + 1], F32, tag="ps_S")
            s_views = [s_lo[:, 0, :], s_lo[:, 1, :], s_hi[:, 0, :], s_hi[:, 1, :]]
            sc4 = small_pool.tile([128, CT], F32, tag="sc4")
            m4 = small_pool.tile([128, CT], F32, tag="m4")
            em4 = small_pool.tile([128, CT], BF16, tag="em4")
            zp = small_pool.tile([128, 1], F32, tag="zp")
            zs = small_pool.tile([128, 1], F32, tag="zs")
            sum4 = small_pool.tile([128, CT], F32, tag="sum4")
            rs4 = small_pool.tile([128, CT], F32, tag="rs4", name=f"rs4_{b}")
            st["rs4"] = rs4
            expS = small_pool.tile([128, CT, Q], BF16, tag="expS", name=f"expS{b}")
            st["expS"] = expS
            ac_ps = ps_b.tile([128, H], F32, tag="ps_b")

            for t in range(CT):
                tps = ps_ctxT.tile([128, KT, 128], BF16, tag="ps_ctxT")
                for k in range(KT):
                    nc.tensor.transpose(
                        tps[:, k, :],
                        ctx_sb[:, t, k * 128 : (k + 1) * 128],
                        ident,
                    )
                if t % 2 == 0:
                    nc.scalar.copy(
                        out=ctxT_sb[:, :, t * 128 : (t + 1) * 128], in_=tps
                    )
                else:
                    nc.vector.tensor_copy(
                        ctxT_sb[:, :, t * 128 : (t + 1) * 128], tps
                    )
                for k in range(KT):
                    nc.tensor.matmul(
                        s_views[t],
                        lhsT=ctxT_sb[:, k, t * 128 : (t + 1) * 128],
                        rhs=rhs_ext[:, k, :],
                        start=(k == 0),
                        stop=False,
                    )
                nc.tensor.matmul(
                    s_views[t], lhsT=ones_row, rhs=rhs_sq, start=False, stop=True
                )
                nc.vector.tensor_copy(sc4[:, t : t + 1], s_views[t][:, Q : Q + 1])
                nc.vector.reduce_max(
                    out=m4[:, t : t + 1],
                    in_=s_views[t][:, 0:Q],
                    axis=mybir.AxisListType.X,
                )
                nc.scalar.activation(
                    out=expS[:, t, :],
                    in_=s_views[t][:, 0:Q],
                    func=AF.Exp,
                    bias=sc4[:, t : t + 1],
                    accum_out=sum4[:, t : t + 1],
                )
                # em_t = exp(max_q S_t + s_c_t); ac accumulates immediately
                nc.scalar.activation(
                    out=em4[:, t : t + 1],
                    in_=m4[:, t : t + 1],
                    func=AF.Exp,
                    bias=sc4[:, t : t + 1],
                )
                em_b = em4[:, t : t + 1].to_broadcast([128, 128])
                nc.tensor.matmul(
                    ac_ps,
                    lhsT=em_b,
                    rhs=ctx_sb[:, t, :],
                    start=(t == 0),
                    stop=(t == CT - 1),
                )
            nc.vector.reciprocal(rs4, sum4)

            # Z = sum_c em; 1/Z via GpSimd partition all-reduce
            nc.vector.reduce_sum(out=zp, in_=em4, axis=mybir.AxisListType.X)
            nc.gpsimd.partition_all_reduce(
                zs, zp, channels=128, reduce_op=bass_isa.ReduceOp.add
            )
            rz128 = small_pool.tile([128, 1], F32, tag="rz128")
            nc.vector.reciprocal(rz128, zs)
            bc_sb = small_pool.tile([128, H], F32, tag="bc_sb", name=f"bc_sb{b}")
            st["bc_sb"] = bc_sb
            nc.vector.tensor_scalar_mul(out=bc_sb, in0=ac_ps, scalar1=rz128)

        def stage_late(b, st):
            g_v, ctx_sb = st["g_v"], st["ctx_sb"]
            expS, rs4, bc_sb, q_sb = st["expS"], st["rs4"], st["bc_sb"], st["q_sb"]

            # --- attended_query + G2/G3 per c-tile, stores flowing per tile;
            # per-tile aT evacuation keeps tile 0's store off the full-batch
            # transpose barrier. ---
            at_ps = ps_small.tile([Q, C], BF16, tag="ps_misc")
            aT_sb = small_pool.tile([Q, C], BF16, tag="aT_sb")
            g234s = []
            for t in range(CT):
                nc.tensor.transpose(
                    at_ps[:, t * 128 : (t + 1) * 128], expS[:, t, :], ident
                )
                nc.scalar.copy(
                    out=aT_sb[:, t * 128 : (t + 1) * 128],
                    in_=at_ps[:, t * 128 : (t + 1) * 128],
                )
                aq_ps = ps_aq.tile([128, H], F32, tag="ps_aq")
                nc.tensor.matmul(
                    aq_ps,
                    lhsT=aT_sb[:, t * 128 : (t + 1) * 128],
                    rhs=q_sb[:, :],
                    start=True,
                    stop=True,
                )
                g234 = g_pool.tile([128, 3 * H], F32, tag="g234", name=f"g234_{b}_{t}")
                g234s.append(g234)
                # G2 = aq / rowsum  (normalization folded into evacuation);
                # stored on its own so it reaches the DMA queue ~600 ns before
                # the G3 multiply finishes.
                nc.scalar.activation(
                    out=g234[:, 0:H], in_=aq_ps, func=AF.Copy, scale=rs4[:, t : t + 1]
                )
                nc.sync.dma_start(out=g_v[:, t, H : 2 * H], in_=g234[:, 0:H])
                # G3 = ctx * aq
                nc.vector.tensor_mul(
                    out=g234[:, H : 2 * H], in0=ctx_sb[:, t, :], in1=g234[:, 0:H]
                )
                nc.sync.dma_start(out=g_v[:, t, 2 * H : 3 * H], in_=g234[:, H : 2 * H])
                if echo_queue:
                    echo_queue.pop(0)()

            # --- G4 = ctx * ac (needs the batch-global bc_sb) ---
            for t in range(CT):
                g234 = g234s[t]
                nc.vector.tensor_mul(
                    out=g234[:, 2 * H : 3 * H], in0=ctx_sb[:, t, :], in1=bc_sb
                )
                nc.gpsimd.dma_start(
                    out=g_v[:, t, 3 * H : 4 * H], in_=g234[:, 2 * H : 3 * H]
                )

        # All loads up front: they have no dependencies (every batch has its
        # own buffer), so the scheduler can never trap a load behind a
        # compute-gated store on the queue.  Echo chunks: the first N_HEAD
        # go at the head of the SP queue (they interleave into the load
        # phase's descriptor-gen bubbles by arrival order); the rest are
        # emitted one per G23 store so they pace with the store stream.
        sts = {b: stage_load(b) for b in range(BPC)}
        echoes = [
            (lambda b=b, t=t: sts[b]["g1"](t)) for b in range(BPC) for t in range(CT)
        ]
        N_HEAD = 6
        for e in echoes[:N_HEAD]:
            e()
        echo_queue = echoes[N_HEAD:]
        for b in range(BPC):
            stage_early(b, sts[b])
            stage_late(b, sts[b])
        sts.clear()

    nc.compile()
    return nc


_NC_CACHE = None


def kernel(context: np.ndarray, query: np.ndarray, W: np.ndarray) -> np.ndarray:
    global _NC_CACHE
    if _NC_CACHE is None:
        _NC_CACHE = build_nc()
    nc = _NC_CACHE

    context = np.ascontiguousarray(context, dtype=np.float32)
    query = np.ascontiguousarray(query, dtype=np.float32)
    W = np.ascontiguousarray(W, dtype=np.float32)

    in_maps = [
        {
            "context": context[i * BPC : (i + 1) * BPC],
            "query": query[i * BPC : (i + 1) * BPC],
            "W": W,
        }
        for i in range(NCORES)
    ]
    res = run_bass_kernel_spmd(nc, in_maps, core_ids=list(range(NCORES)))
    return np.concatenate([r["G"] for r in res.results], axis=0)
